# revision 11
# baseline (speedup 1.0000x reference)
"""MinibatchDiscrimination kernel for 8 Trainium2 NeuronCores.

ref: M = (x @ T.reshape(IN, OUT*K)).reshape(B, OUT, K)
     norm[i,j,o] = sum_k |M[j,o,k]-M[i,o,k]|
     o_b = exp(-norm).sum(0) - 1            # (B, OUT)
     out = concat([x, o_b], 1)              # (B, IN+OUT)

Device layout ("option B"): M^T tiles (128 = 8 o's x 16 k's partitions,
512 batch free).  Per output row j: one DVE tensor_scalar computes
|M_t - M_t[:, j]| fused (op0=subtract, op1=abs_max vs 0) at bf16 4x; a
PE matmul with a 0/1 selection lhsT reduces the 16 k-partitions per o
and packs 16 j's into one 128-row PSUM bank; one ScalarE activation
does exp(-norm) with accum_out giving the i-sum.

Sharding: j (output rows) sharded 8 ways.  SPMD cores run identical
code; each core's x^T input has its batch columns rotated so that its
own 64 j-columns sit at positions 0..63.  GEMM (tiny) is replicated.
"""

import numpy as np

B, IN, OUT, K = 512, 512, 64, 16
NC = 8
JB = B // NC  # 64 output rows per core
NOT = 8       # number of (o,k) tiles: 8 tiles x (8 o's * 16 k's = 128 partitions)


def _patch_tile_drain():
    """walrus in this container rejects >1 sync-wait on a CTRL (Drain)
    instruction; emit one single-wait NOP per pending proc instead."""
    from concourse import tile
    from concourse.vector_clock import ScopedClock, VectorClock

    if getattr(tile.TileContext, "_drain_patched", False):
        return

    def _drain_and_barrier(self, tick_clock, wait_clock):
        vc = tick_clock.global_clock
        n = len(vc)
        for i in range(n):
            t = vc[i]
            if t == 0:
                continue
            v = VectorClock([0] * n)
            v.require_at_least(i, t)
            nop = self.nc.sync.nop(nofuse=True)
            wait_clock.add_sem_waits(nop.ins, ScopedClock({None: v}))
        self.nc.sync.drain()
        self.nc.all_engine_barrier()
        popped = self.nc._tile_sem_poison_stack.pop()
        assert popped is self._sem_poison
        self.nc.clear_and_free_semaphores(list(self.sems.allocated().values()))
        self.nc.all_engine_barrier()

    tile.TileContext._drain_and_barrier = _drain_and_barrier
    tile.TileContext._drain_patched = True


def _patch_wait_split():
    """walrus in this container allows only 1 sync-wait on several
    instruction structs (CTRL/Drain, Ldweights from Matmult lowering).
    Rewrite the BIR before compile: move every instruction's sync waits
    onto single-wait NoOps inserted immediately before it on the same
    engine."""
    import json as _json
    from concourse import bass_utils, bass2jax

    if getattr(bass_utils, "_wait_split_patched", False):
        return

    orig_compile = bass_utils.compile_bir_kernel

    def _rewrite(bir_json: bytes) -> bytes:
        j = _json.loads(bir_json)
        changed = False
        for fn in j.get("functions", []):
            for bb in fn.get("blocks", []):
                new_insts = []
                for inst in bb.get("instructions", []):
                    si = inst.get("sync_info") or {}
                    ow = si.get("on_wait") or []
                    if len(ow) > 1:
                        changed = True
                        for k, w in enumerate(ow):
                            new_insts.append({
                                "debug": inst.get("debug", 0),
                                "engine": inst["engine"],
                                "ins": [],
                                "name": f"{inst['name']}_w{k}",
                                "opcode": "NoOp",
                                "outs": [],
                                "sync_info": {"on_update": [], "on_wait": [w]},
                            })
                        si["on_wait"] = []
                    new_insts.append(inst)
                bb["instructions"] = new_insts
        if not changed:
            return bir_json
        return _json.dumps(j).encode()

    def compile_bir_kernel(bir_json, tmpdir, neff_name="file.neff"):
        return orig_compile(_rewrite(bir_json), tmpdir, neff_name)

    bass_utils.compile_bir_kernel = compile_bir_kernel
    if getattr(bass2jax, "compile_bir_kernel", None) is not None:
        bass2jax.compile_bir_kernel = compile_bir_kernel
    bass_utils._wait_split_patched = True


_CACHE = {}


def _build():
    if "nc" in _CACHE:
        return _CACHE["nc"]
    _patch_tile_drain()
    _patch_wait_split()
    import ml_dtypes
    from concourse import bass, tile
    import concourse.mybir as mybir

    f32 = mybir.dt.float32
    bf16 = mybir.dt.bfloat16
    Alu = mybir.AluOpType

    nc = bass.Bass("TRN2", target_bir_lowering=False, debug=False, num_devices=NC)

    w_h = nc.declare_dram_parameter("w", [IN, OUT * K], f32, isOutput=False)
    xT_h = nc.declare_dram_parameter("xT", [IN, B], f32, isOutput=False)
    xr_h = nc.declare_dram_parameter("xr", [JB, IN], f32, isOutput=False)
    out_h = nc.declare_dram_parameter("out", [JB, IN + OUT], f32, isOutput=True)
    ob_h = nc.declare_dram_parameter("ob", [128, 32], f32, isOutput=True)

    # selection weights: SEL_jl[r=(o',k), m] = 1 iff m == o'*16 + jl
    sel_np = np.zeros((16, 128, 128), dtype=ml_dtypes.bfloat16)
    for jl in range(16):
        for op in range(8):
            for k in range(K):
                sel_np[jl, op * K + k, op * 16 + jl] = 1.0
    sel_dram = [nc.inline_tensor(sel_np[jl], name=f"sel{jl}") for jl in range(16)]
    seln_dram = [nc.inline_tensor(-sel_np[jl], name=f"seln{jl}") for jl in range(16)]

    with tile.TileContext(nc) as tc:
        with (
            tc.tile_pool(name="persist", bufs=1) as persist,
            tc.tile_pool(name="wp", bufs=3) as wp,
            tc.tile_pool(name="absd", bufs=6) as absd_p,
            tc.tile_pool(name="exps", bufs=2) as exps_p,
            tc.tile_pool(name="pg", bufs=2, space="PSUM") as pg_p,
            tc.tile_pool(name="pn", bufs=2, space="PSUM") as pn_p,
        ):
            # persistent loads
            xt_tiles = []
            for kc in range(4):
                t = persist.tile([128, B], f32, tag=f"xt{kc}")
                nc.sync.dma_start(t[:], xT_h[kc * 128:(kc + 1) * 128, :])
                xt_tiles.append(t)
            sel_tiles = []
            seln_tiles = []
            for jl in range(16):
                t = persist.tile([128, 128], bf16, tag=f"sel{jl}")
                nc.sync.dma_start(t[:], sel_dram[jl][:, :])
                sel_tiles.append(t)
                tn = persist.tile([128, 128], bf16, tag=f"seln{jl}")
                nc.sync.dma_start(tn[:], seln_dram[jl][:, :])
                seln_tiles.append(tn)
            acc = persist.tile([128, 32], f32, tag="acc")
            absmask = persist.tile([128, 1], mybir.dt.uint32, tag="absmask")
            nc.vector.memset(absmask[:], 0x7FFF7FFF)

            # GEMM: M^T tile t rows = w cols [t*128,(t+1)*128), cols = batch
            mts = []
            mtfs = []
            for t in range(NOT):
                pg = pg_p.tile([128, B], f32, tag="gemm")
                for kc in range(4):
                    wt = wp.tile([128, 128], f32, tag="w")
                    nc.sync.dma_start(
                        wt[:], w_h[kc * 128:(kc + 1) * 128, t * 128:(t + 1) * 128]
                    )
                    nc.tensor.matmul(
                        pg[:], lhsT=wt[:], rhs=xt_tiles[kc][:],
                        start=(kc == 0), stop=(kc == 3),
                    )
                mt = persist.tile([128, B], bf16, tag=f"mt{t}")
                nc.scalar.copy(mt[:], pg[:])
                mts.append(mt)
                mtf = persist.tile([128, B], f32, tag=f"mtf{t}")
                nc.vector.tensor_copy(mtf[:], pg[:])
                mtfs.append(mtf)

            # pairwise + k-reduce + exp/i-sum
            for ot in range(NOT):
                for jb in range(4):
                    pn = pn_p.tile([128, B], f32, tag="norm")
                    for jl in range(16):
                        jj = jb * 16 + jl  # local j = batch column jj
                        mn = absd_p.tile([128, B], bf16, tag="mn")
                        nc.vector.tensor_scalar(
                            mn[:], mts[ot][:], mtfs[ot][:, jj:jj + 1], None,
                            op0=Alu.min,
                        )
                        nc.tensor.matmul(
                            pn[:], lhsT=sel_tiles[jl][:], rhs=mn[:],
                            start=(jl == 0), stop=False,
                        )
                        mx = absd_p.tile([128, B], bf16, tag="mx")
                        nc.vector.tensor_scalar(
                            mx[:], mts[ot][:], mtfs[ot][:, jj:jj + 1], None,
                            op0=Alu.max,
                        )
                        nc.tensor.matmul(
                            pn[:], lhsT=seln_tiles[jl][:], rhs=mx[:],
                            start=False, stop=(jl == 15),
                        )
                    es = exps_p.tile([128, B], bf16, tag="exps")
                    col = ot * 4 + jb
                    nc.scalar.activation(
                        es[:], pn[:], mybir.ActivationFunctionType.Exp,
                        scale=1.0, accum_out=acc[:, col:col + 1],
                    )

            # o_b = acc - 1 ; ship raw (128, 32); host reorders
            nc.vector.tensor_scalar_add(acc[:], acc[:], -1.0)
            nc.sync.dma_start(ob_h[:, :], acc[:])
            # x passthrough rows for this core
            nc.sync.dma_start(out_h[:, 0:IN], xr_h[:, :])

    nc.finalize()
    _CACHE["nc"] = nc
    return nc


def kernel(x, T):
    from concourse.bass_utils import run_bass_kernel_spmd

    x = np.asarray(x, dtype=np.float32)
    T = np.asarray(T, dtype=np.float32)
    w = np.ascontiguousarray(T.reshape(IN, OUT * K))
    xT = np.ascontiguousarray(x.T)

    nc = _build()
    in_maps = []
    for c in range(NC):
        perm = np.concatenate([np.arange(c * JB, B), np.arange(0, c * JB)])
        in_maps.append({
            "w": w,
            "xT": np.ascontiguousarray(xT[:, perm]),
            "xr": np.ascontiguousarray(x[c * JB:(c + 1) * JB]),
        })
    res = run_bass_kernel_spmd(nc, in_maps, list(range(NC))).results

    out = np.empty((B, IN + OUT), dtype=np.float32)
    for c in range(NC):
        blk = res[c]["out"]            # (JB, IN+OUT): x part filled on device
        ob = res[c]["ob"]              # (128, 32): [o'*16+jl, ot*4+jb]
        o_b = ob.reshape(8, 16, 8, 4).transpose(3, 1, 2, 0).reshape(JB, OUT)
        blk = blk.copy()
        blk[:, IN:] = o_b
        out[c * JB:(c + 1) * JB] = blk
    return out
